# revision 18
# baseline (speedup 1.0000x reference)
"""CFG dual cross-attention on 8 Trainium2 NeuronCores (Bass/Tile).

Sharding: the cfg axis (cond/uncond) splits the 8 cores into 2 groups of 4;
within a group the 4096 query rows are sharded 4-way (1024 rows/core) and the
K/V projection is sharded 4-way over heads.  Each core computes K^T/V for its
10 heads, the group AllGathers K/V (plus exact partial sum-of-squares rows for
the K rms-norm), and every core then runs all 40 heads of attention over its
own query rows.  The host concatenates the row shards.

Matmul operands are bf16 (fp32 PSUM accumulation); softmax/rms statistics in
fp32.  All weights are repacked host-side so every streamed weight tile is a
single fully-contiguous DMA read, and hT/cT are packed per-chunk contiguous.

Attention uses the transposed-logits formulation: logits [L-part, s-free] per
head, exp on the scalar engine, key-axis sum via ones-matmul, softmax 1/sum
via the fast approx reciprocal.  The head loop is software-pipelined: QK(h)
and exp(h) issue before sum/AV(h-1), so the exp latency hides under 12 PE
matmuls and the PE never re-enters the HAM cold state mid-chunk.  All
attention-critical DMAs run on the sync/vector/gpsimd queues (the scalar
queue is saturated by exp), q^T chunk readbacks are split into 8-head pieces
scaled as they land, the first two s-chunks of the attention output are
written directly to SBUF, and the output projection runs in two s-halves so
its first matmul needs no post-attention DMA.
"""

from contextlib import ExitStack

import numpy as np

import concourse.bass as bass
import concourse.bacc as bacc
import concourse.mybir as mybir
import concourse.tile as tile
from concourse import bass_utils

EPS = 1e-6
F32 = mybir.dt.float32
F32R = mybir.dt.float32r

# ---- problem shape (nn_CFGDualCrossAttention: D=5120, H=40, S=4096, L=512) ----
D = 5120
L = 512
S_SHARD = 1024        # 4096 / 4 cores per cfg group
KO = D // 128         # contraction subtiles == heads (head_dim 128)
H = KO
LSUB = L // 128
QCH = 512             # q projection chunk (2 per shard)
SCH = 256             # attention sub-chunk (2 per q chunk)
NSUB = S_SHARD // SCH
R = 4                 # cores per cfg group
MSH = KO // R         # kv-shard m-tiles (10)
VSH = D // R          # kv-shard output cols (1280)
MM = mybir.dt.bfloat16
SCALE = float(128 ** -0.5)

# AllGather buffer layout (bf16 elements)
K_ELEMS = MSH * 128 * L           # 655360
SS_ELEMS = L                      # 512  (partial sum-of-squares row)
V_ELEMS = LSUB * 128 * VSH        # 655360
SHARD_ELEMS = K_ELEMS + SS_ELEMS + V_ELEMS

TRACE = False         # set by test harness for NTFF timing
LAST_EXEC_NS = None
_CACHED_NC = None


def _build() -> bacc.Bacc:
    mm = MM
    WKT = 5           # wk/wv stream tiles (256 cols each)
    WOT = D // 256    # wo stream tiles

    nc = bacc.Bacc("TRN2", target_bir_lowering=False, debug=False, num_devices=8)

    # ---- external inputs (host-side repacked; see kernel() below) ----
    hT_p = nc.dram_tensor("hT_p", [128, KO * S_SHARD], mm,
                          kind="ExternalInput")
    cT_p = nc.dram_tensor("cT_p", [128, KO * L], mm, kind="ExternalInput")
    wq_p = nc.dram_tensor("wq_p", [KO, 128, KO * 128], mm, kind="ExternalInput")
    wk_p = nc.dram_tensor("wk_p", [WKT, 128, KO * 256], mm, kind="ExternalInput")
    wv_p = nc.dram_tensor("wv_p", [WKT, 128, KO * 256], mm, kind="ExternalInput")
    wo_p = nc.dram_tensor("wo_p", [WOT, 128, KO * 256], mm, kind="ExternalInput")
    gq_pm = nc.dram_tensor("gq_pm", [128, KO], F32, kind="ExternalInput")
    bqgq_pm = nc.dram_tensor("bqgq_pm", [128, KO], F32, kind="ExternalInput")
    gk_pm = nc.dram_tensor("gk_pm", [128, MSH], F32, kind="ExternalInput")
    bkgk_pm = nc.dram_tensor("bkgk_pm", [128, MSH], F32, kind="ExternalInput")
    bv_sh = nc.dram_tensor("bv_sh", [VSH], F32, kind="ExternalInput")
    bot = nc.dram_tensor("bo", [D], F32, kind="ExternalInput")
    out = nc.dram_tensor("out", [S_SHARD, D], mm, kind="ExternalOutput")

    oT_dram = nc.dram_tensor("oT_spill", [D, S_SHARD], mm)
    qT_dram = nc.dram_tensor("qT_spill", [KO, 128, S_SHARD], mm)
    kv_in = nc.dram_tensor("kv_in", [SHARD_ELEMS], mm)
    # note: Shared addr_space needs >4-core groups; Local costs one extra copy
    kv_out = nc.dram_tensor("kv_out", [R * SHARD_ELEMS], mm)

    oT_r = oT_dram.rearrange("(ko p) s -> p ko s", p=128)
    out_r = out.rearrange("(cs p) n -> p cs n", p=128)

    replica_groups = [[0, 1, 2, 3], [4, 5, 6, 7]]

    def wdma(i, dst, src):
        # alternate big streaming DMAs across the two HWDGE queues
        (nc.sync if i % 2 == 0 else nc.scalar).dma_start(dst, src)

    def wdma2(dst, src):
        # split one weight tile across both HWDGE queues (halved latency)
        half = dst.shape[1] // 2
        nc.sync.dma_start(dst[:, :half], src[:, :half])
        nc.scalar.dma_start(dst[:, half:], src[:, half:])

    with tile.TileContext(nc) as tc, ExitStack() as top:
        consts = top.enter_context(tc.tile_pool(name="consts", bufs=1))
        gq_sb = consts.tile([128, KO], F32)
        bqgq_sb = consts.tile([128, KO], F32)
        gk_sb = consts.tile([128, MSH], F32)
        bkgk_sb = consts.tile([128, MSH], F32)
        ones_sb = consts.tile([128, 1], mm)
        ones4 = consts.tile([4, 1], mm)
        eps_sb = consts.tile([1, 1], F32)
        eps128_sb = consts.tile([1, 1], F32)
        nc.scalar.dma_start(gq_sb, gq_pm.ap())
        nc.scalar.dma_start(bqgq_sb, bqgq_pm.ap())
        nc.scalar.dma_start(gk_sb, gk_pm.ap())
        nc.scalar.dma_start(bkgk_sb, bkgk_pm.ap())
        nc.vector.memset(ones_sb, 1.0)
        nc.vector.memset(ones4, 1.0)
        nc.vector.memset(eps_sb, EPS)
        nc.vector.memset(eps128_sb, 128.0 * EPS)

        # k^T and v (full, gathered) live across attention; freed before Oproj
        with ExitStack() as acts_scope:
            act_pool = acts_scope.enter_context(tc.tile_pool(name="acts", bufs=1))
            kT_sb = act_pool.tile([128, KO, L], mm, tag="kT")
            v_sb = act_pool.tile([128, LSUB, D], mm)
            kinv_rep = act_pool.tile([128, L], F32, name="kinv_rep")
            ss4_sb = act_pool.tile([4, L], mm, name="ss4")

            # =========== K + V shard (this core's 10 heads) ===========
            with ExitStack() as ph:
                cpool = ph.enter_context(tc.tile_pool(name="ctx", bufs=1))
                wpool = ph.enter_context(tc.tile_pool(name="wkv", bufs=2))
                spool = ph.enter_context(tc.tile_pool(name="kscr", bufs=2))
                pp_mm = ph.enter_context(tc.tile_pool(name="ppkv", bufs=2,
                                                      space="PSUM"))
                pp_ss = ph.enter_context(tc.tile_pool(name="ppkss", bufs=1,
                                                      space="PSUM"))

                cT_sb = cpool.tile([128, KO, L], mm)
                cT_r = cT_p.rearrange("p (ko l) -> p ko l", ko=KO)
                # latency-ordered start: wk0 first (both queues), then the cT
                # quarters each split across both queues so the first K
                # matmuls start ~6us in and stream behind the quarters
                wk_tiles = []
                wk_sb = wpool.tile([128, KO, 256], mm, tag="w", name="wk_sb")
                wdma2(wk_sb, wk_p.ap()[0].rearrange("p (ko c) -> p ko c", ko=KO))
                wk_tiles.append(wk_sb)
                wdma2(cT_sb[:, bass.ts(0, 10), :], cT_r[:, bass.ts(0, 10), :])
                wdma2(cT_sb[:, bass.ts(1, 10), :], cT_r[:, bass.ts(1, 10), :])
                wk_sb = wpool.tile([128, KO, 256], mm, tag="w", name="wk_sb")
                wdma2(wk_sb, wk_p.ap()[1].rearrange("p (ko c) -> p ko c", ko=KO))
                wk_tiles.append(wk_sb)
                wdma2(cT_sb[:, bass.ts(2, 10), :], cT_r[:, bass.ts(2, 10), :])
                wdma2(cT_sb[:, bass.ts(3, 10), :], cT_r[:, bass.ts(3, 10), :])
                bv_rep = cpool.tile([128, VSH], mm, name="bv_rep")
                nc.gpsimd.dma_start(bv_rep,
                                    bv_sh.ap()[None, :].to_broadcast([128, VSH]))
                kTs = cpool.tile([128, MSH, L], mm, name="kTs")

                ss_ps = pp_ss.tile([128, 512], F32, name="ps_kss")
                sq_prev = None
                for t in range(WKT):
                    if t < 2:
                        wk_sb = wk_tiles[t]
                    else:
                        wk_sb = wpool.tile([128, KO, 256], mm, tag="w",
                                           name="wk_sb")
                        wdma2(wk_sb,
                              wk_p.ap()[t].rearrange("p (ko c) -> p ko c",
                                                     ko=KO))
                    for mi in range(2):
                        m = 2 * t + mi
                        ps = pp_mm.tile([128, 512], F32, tag="mm", name="ps_k")
                        for ko in range(KO):
                            nc.tensor.matmul(ps, wk_sb[:, ko, bass.ts(mi, 128)],
                                             cT_sb[:, ko, :],
                                             start=(ko == 0), stop=(ko == KO - 1))
                        # k~ = gk*(Wk c + bk): fused scale+bias eviction
                        nc.scalar.activation(kTs[:, m, :], ps,
                                             mybir.ActivationFunctionType.Identity,
                                             bias=bkgk_sb[:, m:m + 1],
                                             scale=gk_sb[:, m:m + 1])
                        sq = spool.tile([128, 512], mm, tag="sq", name="sq")
                        nc.vector.tensor_mul(sq, kTs[:, m, :], kTs[:, m, :])
                        # delayed by one m so the evict->square chain never
                        # stalls the PE stream
                        if sq_prev is not None:
                            nc.tensor.matmul(ss_ps[:1, :L], ones_sb, sq_prev,
                                             start=(m == 1), stop=False)
                        sq_prev = sq
                nc.tensor.matmul(ss_ps[:1, :L], ones_sb, sq_prev,
                                 start=False, stop=True)
                ssk_bf = cpool.tile([1, L], mm, name="ssk_bf")
                nc.scalar.activation(ssk_bf, ss_ps[:1, :L],
                                     mybir.ActivationFunctionType.Copy)
                # spill K~^T shard + partial ss row into the AG input buffer
                nc.scalar.dma_start(
                    kv_in.ap()[:K_ELEMS].rearrange("(m p l) -> p m l",
                                                   m=MSH, p=128, l=L), kTs)
                nc.scalar.dma_start(
                    kv_in.ap()[K_ELEMS:K_ELEMS + SS_ELEMS][None, :], ssk_bf)

                # ---- V shard ----
                vs = cpool.tile([128, LSUB, VSH], mm, name="vs")
                for t in range(WKT):
                    wv_sb = wpool.tile([128, KO, 256], mm, tag="w", name="wv_sb")
                    wdma2(wv_sb,
                          wv_p.ap()[t].rearrange("p (ko c) -> p ko c", ko=KO))
                    for lb in range(LSUB):
                        ps = pp_mm.tile([128, 512], F32, tag="mm",
                                        name="ps_v")[:, :256]
                        for ko in range(KO):
                            nc.tensor.matmul(ps, cT_sb[:, ko, bass.ts(lb, 128)],
                                             wv_sb[:, ko, :],
                                             start=(ko == 0), stop=(ko == KO - 1))
                        nc.vector.tensor_add(vs[:, lb, bass.ts(t, 256)], ps,
                                             bv_rep[:, bass.ts(t, 256)])
                # split across both queues: this spill completes only at the
                # V-proj tail, and whichever queue carries it stalls its
                # share of the Q-phase prefetch behind it
                vdst = kv_in.ap()[K_ELEMS + SS_ELEMS:].rearrange(
                    "(lb p n) -> p lb n", lb=LSUB, p=128, n=VSH)
                nc.sync.dma_start(vdst[:, :2, :], vs[:, :2, :])
                nc.scalar.dma_start(vdst[:, 2:, :], vs[:, 2:, :])

            # =========== AllGather K/V within each cfg group ===========
            nc.gpsimd.collective_compute(
                "AllGather", mybir.AluOpType.bypass,
                replica_groups=replica_groups,
                ins=[kv_in.ap()], outs=[kv_out.ap()])
            for r in range(R):
                base = r * SHARD_ELEMS
                nc.gpsimd.dma_start(
                    kT_sb[:, r * MSH:(r + 1) * MSH, :],
                    kv_out.ap()[base:base + K_ELEMS].rearrange(
                        "(m p l) -> p m l", m=MSH, p=128, l=L))
                nc.gpsimd.dma_start(
                    v_sb[:, :, r * VSH:(r + 1) * VSH],
                    kv_out.ap()[base + K_ELEMS + SS_ELEMS:base + SHARD_ELEMS]
                    .rearrange("(lb p n) -> p lb n", lb=LSUB, p=128, n=VSH))
            nc.gpsimd.dma_start(
                ss4_sb,
                kv_out.ap().rearrange("(r x) -> r x", r=R)[:, K_ELEMS:K_ELEMS +
                                                           SS_ELEMS])

            # ===== Q projection: one pass over Wq, q^T spilled to DRAM =====
            qsc_a = act_pool.tile([1, QCH], F32, name="qsc_a")
            qsc_b = act_pool.tile([1, QCH], F32, name="qsc_b")
            qsc_h = [qsc_a, qsc_b]
            with ExitStack() as qproj:
                hpool = qproj.enter_context(tc.tile_pool(name="hq", bufs=1))
                wpool = qproj.enter_context(tc.tile_pool(name="wq", bufs=2))
                qmpool = qproj.enter_context(tc.tile_pool(name="qtm", bufs=3))
                spool = qproj.enter_context(tc.tile_pool(name="qscr", bufs=2))
                spool1 = qproj.enter_context(tc.tile_pool(name="qscr1",
                                                          bufs=1))
                pp_mm = qproj.enter_context(
                    tc.tile_pool(name="ppmmq", bufs=2, space="PSUM"))
                pp_ss = qproj.enter_context(
                    tc.tile_pool(name="ppqss", bufs=1, space="PSUM"))

                hT_sb = hpool.tile([128, KO, S_SHARD], mm)
                hT_r = hT_p.rearrange("p (ko s) -> p ko s", ko=KO)
                # latency-ordered start: wq0 first (split), then hT quarters
                # each split across both queues; first Q matmul ~8us after
                # the KV spills drain, streaming behind the quarters
                wq_tiles = []
                wq_sb = wpool.tile([128, KO, 128], mm, tag="w", name="wq_sb")
                wdma2(wq_sb, wq_p.ap()[0].rearrange("p (ko c) -> p ko c", ko=KO))
                wq_tiles.append(wq_sb)
                wdma2(hT_sb[:, bass.ts(0, 10), :], hT_r[:, bass.ts(0, 10), :])
                wdma2(hT_sb[:, bass.ts(1, 10), :], hT_r[:, bass.ts(1, 10), :])
                wq_sb = wpool.tile([128, KO, 128], mm, tag="w", name="wq_sb")
                wdma2(wq_sb, wq_p.ap()[1].rearrange("p (ko c) -> p ko c", ko=KO))
                wq_tiles.append(wq_sb)
                wdma2(hT_sb[:, bass.ts(2, 10), :], hT_r[:, bass.ts(2, 10), :])
                wdma2(hT_sb[:, bass.ts(3, 10), :], hT_r[:, bass.ts(3, 10), :])
                ss_a = pp_ss.tile([128, 512], F32, name="ps_qss_a")
                ss_b = pp_ss.tile([128, 512], F32, name="ps_qss_b")
                ss_h = [ss_a, ss_b]
                sq_prev = [None, None]
                for m in range(KO):
                    if m < 2:
                        wq_sb = wq_tiles[m]
                    else:
                        wq_sb = wpool.tile([128, KO, 128], mm, tag="w",
                                           name="wq_sb")
                        wdma(m, wq_sb, wq_p.ap()[m].rearrange(
                            "p (ko c) -> p ko c", ko=KO))
                    qTm = qmpool.tile([128, 2, QCH], mm, tag="qtm", name="qTm")
                    for hf in range(2):
                        ps = pp_mm.tile([128, 512], F32, tag="mm", name="ps_q")
                        for ko in range(KO):
                            nc.tensor.matmul(
                                ps, wq_sb[:, ko, :],
                                hT_sb[:, ko, bass.ts(hf, QCH)],
                                start=(ko == 0), stop=(ko == KO - 1))
                        nc.scalar.activation(
                            qTm[:, hf, :], ps,
                            mybir.ActivationFunctionType.Identity,
                            bias=bqgq_sb[:, m:m + 1], scale=gq_sb[:, m:m + 1])
                        sq = spool.tile([128, 512], mm, tag=f"sq{hf}",
                                        name="sq")
                        nc.vector.tensor_mul(sq, qTm[:, hf, :], qTm[:, hf, :])
                        if sq_prev[hf] is not None:
                            nc.tensor.matmul(ss_h[hf][:1, :QCH], ones_sb,
                                             sq_prev[hf],
                                             start=(m == 1), stop=False)
                        sq_prev[hf] = sq
                    wdma(m, qT_dram.ap()[m], qTm)
                    if m == 25:
                        # kinv from the AG'd exact partial ss rows — mid-Q
                        # so the collective is long done and the tiny PE op
                        # never stalls the stream
                        ps4 = pp_mm.tile([128, 512], F32, tag="mm",
                                         name="ps4")[:1, :L]
                        nc.tensor.matmul(ps4, ones4, ss4_sb,
                                         start=True, stop=True)
                        kroot = spool1.tile([1, L], F32, name="kroot",
                                            tag="kroot")
                        nc.scalar.activation(
                            kroot, ps4, mybir.ActivationFunctionType.Sqrt,
                            scale=1.0 / D, bias=eps_sb)
                        kinv = spool1.tile([1, L], F32, name="kinv",
                                           tag="kinv")
                        nc.vector.reciprocal_approx_fast(kinv, kroot)
                        nc.gpsimd.partition_broadcast(kinv_rep, kinv)
                        for g in range(KO // 8):
                            nc.vector.tensor_mul(
                                kT_sb[:, bass.ts(g, 8), :],
                                kT_sb[:, bass.ts(g, 8), :],
                                kinv_rep[:, None, :].to_broadcast([128, 8, L]))
                for hf in range(2):
                    nc.tensor.matmul(ss_h[hf][:1, :QCH], ones_sb, sq_prev[hf],
                                     start=False, stop=True)
                    # qsc = scale / rms(q) per s column (scale folded into
                    # the sqrt)
                    qroot = spool1.tile([1, QCH], F32, name="qroot",
                                        tag="qsc")
                    nc.scalar.activation(qroot, ss_h[hf][:1, :QCH],
                                         mybir.ActivationFunctionType.Sqrt,
                                         scale=128.0 / D, bias=eps128_sb)
                    nc.vector.reciprocal_approx_fast(qsc_h[hf], qroot)

            # ===== attention + output projection (tail scope; pools are a
            # strict stack, so everything the O-projection reads is entered
            # here, where hT_sb's 80 KB has just freed) =====
            with ExitStack() as tail:
                otpool0 = tail.enter_context(tc.tile_pool(name="oth0",
                                                          bufs=1))
                wprepool = tail.enter_context(tc.tile_pool(name="wopre",
                                                           bufs=1))
                # s-chunks 0-1 of the attention output land here directly
                oT_h0 = otpool0.tile([128, KO, 2 * SCH], mm, name="oT_h0")
                with ExitStack() as at_scope:
                    qcpool = at_scope.enter_context(
                        tc.tile_pool(name="qtc", bufs=2))
                    rpool = at_scope.enter_context(
                        tc.tile_pool(name="qrep", bufs=2))
                    spool = at_scope.enter_context(
                        tc.tile_pool(name="ascr", bufs=2))
                    apool = at_scope.enter_context(
                        tc.tile_pool(name="attn", bufs=2))
                    opool = at_scope.enter_context(
                        tc.tile_pool(name="oev", bufs=2))
                    pp_pt = at_scope.enter_context(
                        tc.tile_pool(name="pppt", bufs=2, space="PSUM"))
                    pp_sr = at_scope.enter_context(
                        tc.tile_pool(name="ppsr", bufs=2, space="PSUM"))
                    pp_o = at_scope.enter_context(
                        tc.tile_pool(name="ppo", bufs=2, space="PSUM"))
                    qT_rd = qT_dram.rearrange("m p s -> p m s")

                    # q^T chunk prep: the scalar queue is saturated by exp
                    # during attention, so the readbacks go on sync/gpsimd
                    # in 8-head pieces; each piece's scale-mul is emitted a
                    # few heads after its DMA trigger so the strict-FIFO DVE
                    # never blocks on an in-flight transfer
                    PQ = [nc.sync, nc.gpsimd, nc.sync, nc.gpsimd, nc.sync]

                    def prep_alloc(s0):
                        qTc = qcpool.tile([128, KO, SCH], mm, tag="qtc",
                                          name="qTc")
                        qsc_rep = rpool.tile([128, SCH], F32, tag="qr",
                                             name="qsc_rep")
                        nc.gpsimd.partition_broadcast(
                            qsc_rep, qsc_h[s0 // 2][:, bass.ts(s0 % 2, SCH)])
                        return qTc, qsc_rep

                    def prep_dma(qTc, s0, g):
                        csl = bass.ts(s0, SCH)
                        hsl = bass.ts(g, 8)
                        PQ[g].dma_start(qTc[:, hsl, :], qT_rd[:, hsl, csl])

                    def prep_scale(qTc, qsc_rep, g):
                        hsl = bass.ts(g, 8)
                        nc.vector.tensor_mul(
                            qTc[:, hsl, :], qTc[:, hsl, :],
                            qsc_rep[:, None, :].to_broadcast([128, 8, SCH]))

                    def sum_av(probsT, h, s0):
                        sr = pp_sr.tile([128, 512], F32, tag="sr", name="sr")
                        for lb in range(LSUB):
                            nc.tensor.matmul(sr[:1, :SCH], ones_sb,
                                             probsT[:, lb, :],
                                             start=(lb == 0),
                                             stop=(lb == LSUB - 1))
                        rinv = spool.tile([1, SCH], F32, tag="rinv",
                                          name="rinv")
                        nc.vector.reciprocal_approx_fast(rinv, sr[:1, :SCH])
                        ops = pp_o.tile([128, SCH], F32, tag="o", name="ops")
                        for lb in range(LSUB):
                            nc.tensor.matmul(ops, v_sb[:, lb, bass.ts(h, 128)],
                                             probsT[:, lb, :],
                                             start=(lb == 0),
                                             stop=(lb == LSUB - 1))
                        # replicate 1/sum across partitions off the PE
                        # stream (gpsimd is otherwise idle here)
                        rrep = spool.tile([128, SCH], F32, tag="rrep",
                                          name="rrep")
                        nc.gpsimd.partition_broadcast(rrep, rinv)
                        if s0 < 2:
                            nc.vector.tensor_mul(
                                oT_h0[:, h, bass.ts(s0, SCH)], ops, rrep)
                        else:
                            o_h = opool.tile([128, SCH], mm, tag="oh",
                                             name="o_h")
                            nc.vector.tensor_mul(o_h, ops, rrep)
                            nc.sync.dma_start(oT_r[:, h, bass.ts(s0, SCH)],
                                              o_h)

                    cur = prep_alloc(0)
                    for g in range(5):
                        prep_dma(cur[0], 0, g)
                    for g in range(5):
                        prep_scale(cur[0], cur[1], g)
                    # wo tile 0 prefetch for the O-projection (behind the
                    # chunk-0 pieces: lands ~10us in, needed ~300us later;
                    # only one fits next to the attention working set)
                    wo_pre = wprepool.tile([128, KO, 256], mm, tag="wo",
                                           name="wo_pre")
                    nc.sync.dma_start(
                        wo_pre, wo_p.ap()[0].rearrange("p (ko c) -> p ko c",
                                                       ko=KO))

                    nxt = None
                    prev = None   # (probsT, h, s0) one head behind the PE
                    for s0 in range(NSUB):
                        qTc = cur[0]
                        for h in range(H):
                            if s0 + 1 < NSUB:
                                if h == 2:
                                    nxt = prep_alloc(s0 + 1)
                                if h in (2, 4, 6, 8, 10):
                                    prep_dma(nxt[0], s0 + 1, (h - 2) // 2)
                                if h in (5, 7, 9, 11, 13):
                                    prep_scale(nxt[0], nxt[1], (h - 5) // 2)
                            pt = pp_pt.tile([128, LSUB, SCH], F32, tag="pt",
                                            name="pt")
                            for lb in range(LSUB):
                                nc.tensor.matmul(
                                    pt[:, lb, :],
                                    kT_sb[:, h, bass.ts(lb, 128)],
                                    qTc[:, h, :],
                                    start=(lb % 2 == 0), stop=(lb % 2 == 1))
                            probsT = apool.tile([128, LSUB, SCH], mm,
                                                tag="probsT", name="probsT")
                            nc.scalar.activation(
                                probsT, pt,
                                mybir.ActivationFunctionType.Exp)
                            # sum/AV of the previous head: exp(h) hides
                            # under these 8 matmuls + the next head's QK
                            if prev is not None:
                                sum_av(*prev)
                            prev = (probsT, h, s0)
                        cur = nxt
                    sum_av(*prev)

                # ===== output projection: two s-halves =====
                spool = tail.enter_context(tc.tile_pool(name="oscr", bufs=3))
                wopool = tail.enter_context(tc.tile_pool(name="wo", bufs=2))
                otpool1 = tail.enter_context(tc.tile_pool(name="oth1",
                                                          bufs=1))
                pp_mm = tail.enter_context(tc.tile_pool(name="ppmmo", bufs=2,
                                                        space="PSUM"))
                # kT_sb is dead after attention and has the exact same
                # shape: reuse its buffer (tag round-robin adds the WAR dep)
                oT_h1 = act_pool.tile([128, KO, L], mm, tag="kT",
                                      name="oT_h1")
                bo_rep = otpool1.tile([128, D], mm, name="bo_rep")
                nc.gpsimd.dma_start(bo_rep,
                                    bot.ap()[None, :].to_broadcast([128, D]))
                # chunks 2-3 readback; streams while half 0 computes
                nc.sync.dma_start(oT_h1[:, :20, :],
                                  oT_r[:, :20, 2 * SCH:])
                nc.scalar.dma_start(oT_h1[:, 20:, :],
                                    oT_r[:, 20:, 2 * SCH:])

                pairs = [(half, t) for half in range(2)
                         for t in range(WOT)]
                nxt_wo = wo_pre
                for idx, (half, t) in enumerate(pairs):
                    wo_sb = nxt_wo
                    if idx + 1 < len(pairs):
                        nxt_wo = wopool.tile([128, KO, 256], mm, tag="wo",
                                             name="wo_sb")
                        wdma(idx, nxt_wo,
                             wo_p.ap()[pairs[idx + 1][1]].rearrange(
                                 "p (ko c) -> p ko c", ko=KO))
                    oTh = oT_h0 if half == 0 else oT_h1
                    for cs in range(4):
                        ps = pp_mm.tile([128, 512], F32, tag="mm",
                                        name="ps_o")[:, :256]
                        for ko in range(KO):
                            nc.tensor.matmul(ps,
                                             oTh[:, ko, bass.ts(cs, 128)],
                                             wo_sb[:, ko, :],
                                             start=(ko == 0),
                                             stop=(ko == KO - 1))
                        o_sb = spool.tile([128, 512], mm, tag="out",
                                          name="o_sb")[:, :256]
                        nc.vector.tensor_add(o_sb, ps,
                                             bo_rep[:, bass.ts(t, 256)])
                        nc.scalar.dma_start(
                            out_r[:, 4 * half + cs, bass.ts(t, 256)], o_sb)

    nc.compile()
    return nc


def _get_nc():
    global _CACHED_NC
    if _CACHED_NC is None:
        _CACHED_NC = _build()
    return _CACHED_NC


def _pack_w(wT, tc):
    """[D, N] (contraction-major transposed weight) -> [N//tc, 128, KO*tc]
    so each streamed tile is one fully-contiguous DMA read."""
    n = wT.shape[1]
    nt = n // tc
    return np.ascontiguousarray(
        wT.reshape(KO, 128, nt, tc).transpose(2, 1, 0, 3).reshape(
            nt, 128, KO * tc))


def kernel(hidden_cond, hidden_uncond, context_cond, context_uncond,
           Wq, bq, Wkv, bkv, gq, gk, Wo, bo):
    global LAST_EXEC_NS
    import ml_dtypes
    bf = ml_dtypes.bfloat16 if MM == mybir.dt.bfloat16 else np.float32
    f32 = np.float32

    nc = _get_nc()

    hid = [np.asarray(hidden_cond, f32).reshape(-1, D),
           np.asarray(hidden_uncond, f32).reshape(-1, D)]
    ctxs = [np.asarray(context_cond, f32).reshape(-1, D),
            np.asarray(context_uncond, f32).reshape(-1, D)]
    Wq = np.asarray(Wq, f32)
    Wkv = np.asarray(Wkv, f32)
    Wo = np.asarray(Wo, f32)
    bq = np.asarray(bq, f32)
    bkv = np.asarray(bkv, f32)
    bo = np.asarray(bo, f32)
    gq = np.asarray(gq, f32)
    gk = np.asarray(gk, f32)
    bk, bv = bkv[:D], bkv[D:]

    wq_pk = _pack_w(np.ascontiguousarray(Wq.T).astype(bf), 128)
    wo_pk = _pack_w(np.ascontiguousarray(Wo.T).astype(bf), 256)
    WkT = np.ascontiguousarray(Wkv[:D].T).astype(bf)
    WvT = np.ascontiguousarray(Wkv[D:].T).astype(bf)
    wk_pks = [_pack_w(WkT[:, r * VSH:(r + 1) * VSH], 256) for r in range(R)]
    wv_pks = [_pack_w(WvT[:, r * VSH:(r + 1) * VSH], 256) for r in range(R)]

    common = {
        "wq_p": wq_pk, "wo_p": wo_pk,
        "gq_pm": np.ascontiguousarray(gq.reshape(KO, 128).T),
        "bqgq_pm": np.ascontiguousarray((bq * gq).reshape(KO, 128).T),
        "bo": bo,
    }
    cT_ps = []
    for g in range(2):
        cT = np.ascontiguousarray(ctxs[g].T).astype(bf)   # [D, L]
        cT_ps.append(np.ascontiguousarray(
            cT.reshape(KO, 128, L).transpose(1, 0, 2).reshape(128, KO * L)))

    in_maps = []
    for core in range(8):
        g, r = core // 4, core % 4
        hT = np.ascontiguousarray(
            hid[g][r * S_SHARD:(r + 1) * S_SHARD].T).astype(bf)  # [D, S_SHARD]
        hT_pk = np.ascontiguousarray(
            hT.reshape(KO, 128, S_SHARD).transpose(1, 0, 2)
            .reshape(128, KO * S_SHARD))
        sl = slice(r * VSH, (r + 1) * VSH)
        in_maps.append({
            "hT_p": hT_pk, "cT_p": cT_ps[g],
            "wk_p": wk_pks[r], "wv_p": wv_pks[r],
            "gk_pm": np.ascontiguousarray(gk[sl].reshape(MSH, 128).T),
            "bkgk_pm": np.ascontiguousarray((bk * gk)[sl].reshape(MSH, 128).T),
            "bv_sh": np.ascontiguousarray(bv[sl]),
            **common,
        })

    res = bass_utils.run_bass_kernel_spmd(nc, in_maps, list(range(8)),
                                          trace=TRACE)
    LAST_EXEC_NS = res.exec_time_ns

    out_c = np.concatenate(
        [np.asarray(res.results[i]["out"], f32) for i in range(4)], axis=0)
    out_u = np.concatenate(
        [np.asarray(res.results[i]["out"], f32) for i in range(4, 8)], axis=0)
    return (out_c[None], out_u[None])
